# revision 1
# baseline (speedup 1.0000x reference)
"""Causal self-attention (B=2, T=2048, C=1024, H=16) on 8 trn2 NeuronCores.

Sharding: 16 heads / 8 cores = 2 heads per core (both batches on every core).
Per core, for its head pair (h0 at partitions 0-63, h1 at 64-127):
  - QKV projection of the full sequence (384 weight columns), producing
    qT/kT in [head_dim, T] layout and V' in [T, head_dim] layout via PE
    transposes, with a ones column appended per head (softmax denominator).
  - Flash-style causal attention on-chip: the two heads' S^T tiles are
    computed as concurrent row-group matmuls into one 2-bank PSUM tile,
    one ACT exp covers both heads, causal masking multiplies a triangular
    0/1 mask on diagonal tiles only (GpSimd), att@V accumulates per head
    with the ones column yielding the denominator row.
  - Output projection as concurrent row-group matmul pairs (y_h1 shifted
    to partitions 64-127); softmax normalization is applied on the
    PSUM->SBUF copies as a per-partition (per-query) reciprocal scale,
    transported via a DRAM-bounce transpose of the denominator rows.
Host glue: transpose/round x, slice weights per core, sum the 8 partial
outputs, add b_proj.

Matmuls run in float32r (fp32 with 12-bit mantissa, 4x faster than fp32 on
the PE, full fp32 PSUM accumulate). Operands are pre-rounded on host or
rounded by the producing engine (f32r output APs).
"""

import sys

sys.path.insert(0, "/opt/trn_rl_repo")

import numpy as np

B, T, C, H, HD = 2, 2048, 1024, 16, 64
BT = B * T
NCORE = 8
HPC = H // NCORE  # heads per core
NT = BT // 512    # T-tiles for qkv projection
CCH = C // 128    # contraction chunks


def _round_f32r(x):
    x = np.ascontiguousarray(x, dtype=np.float32)
    xi = x.view(np.uint32)
    r = (xi + np.uint32(0x7FF) + ((xi >> np.uint32(12)) & np.uint32(1))) & np.uint32(
        0xFFFFF000
    )
    return r.view(np.float32)


_CACHE = {}


def _build():
    if "nc" in _CACHE:
        return _CACHE["nc"]
    from contextlib import ExitStack

    import concourse.bass as bass
    import concourse.bacc as bacc
    import concourse.mybir as mybir
    import concourse.tile as tile
    from concourse.masks import make_identity, make_upper_triangular

    f32, f32r = mybir.dt.float32, mybir.dt.float32r
    AF = mybir.ActivationFunctionType
    ALU = mybir.AluOpType

    nc = bacc.Bacc(None, target_bir_lowering=False, debug=False)
    # x pre-permuted on host to the exact SBUF image [p, tt, cc, t] so each
    # T-tile DMA reads 16KB-contiguous runs per partition
    xT_d = nc.dram_tensor("xT", [128, NT, CCH, 512], f32r, kind="ExternalInput")
    wqkv_d = nc.dram_tensor("wqkv", [128, CCH, 3 * 128], f32r, kind="ExternalInput")
    bqkv_d = nc.dram_tensor("bqkv", [128, 3], f32, kind="ExternalInput")
    wp_d = nc.dram_tensor("wp", [128, C], f32r, kind="ExternalInput")
    out_d = nc.dram_tensor("out", [BT, C], f32, kind="ExternalOutput")

    with tile.TileContext(nc) as tc, ExitStack() as ctx:
        sb = ctx.enter_context(tc.tile_pool(name="sb", bufs=1))
        xp = ctx.enter_context(tc.tile_pool(name="xp", bufs=3))
        vtp = ctx.enter_context(tc.tile_pool(name="vtp", bufs=2))
        esp = ctx.enter_context(tc.tile_pool(name="esp", bufs=4))
        ytp = ctx.enter_context(tc.tile_pool(name="ytp", bufs=3))
        dnp = ctx.enter_context(tc.tile_pool(name="dnp", bufs=4))
        outp = ctx.enter_context(tc.tile_pool(name="outp", bufs=3))
        drp = ctx.enter_context(tc.tile_pool(name="drp", bufs=8, space="DRAM"))
        # PSUM: 2 + 4 + 2 = 8 banks
        pa = ctx.enter_context(tc.tile_pool(name="pa", bufs=2, space="PSUM"))
        pss = ctx.enter_context(tc.tile_pool(name="pss", bufs=2, space="PSUM"))
        pso = ctx.enter_context(tc.tile_pool(name="pso", bufs=2, space="PSUM"))

        wq_sb = sb.tile([128, CCH, 3 * 128], f32r, tag="wq")

        qT = [sb.tile([128, T], f32r, tag=f"qT{b}", name=f"qT{b}") for b in range(B)]
        kT = [sb.tile([128, T], f32r, tag=f"kT{b}", name=f"kT{b}") for b in range(B)]
        # V' per batch: per k-tile [V_h0 (64) | 1 | V_h1 (64) | 1] = 130 cols;
        # the ones column accumulates the softmax denominator during att@v
        VW = 130
        Vp = [
            sb.tile([128, 16, VW], f32r, tag=f"Vp{b}", name=f"Vp{b}") for b in range(B)
        ]
        for b in range(B):
            # ones columns (64, 129) and finite pad from blanket 1.0 fill;
            # V columns are overwritten by the transpose copies below
            nc.vector.memset(Vp[b][:, :, :].bitcast(f32), 1.0)

        # ---------------- Phase A: QKV projection ----------------
        x_tiles = []
        for tt in range(NT):
            x_t = xp.tile([128, CCH, 512], f32r, tag="x", name=f"x{tt}")
            if tt == 0:
                # chunked loads so the first matmul chain starts after ~400KB
                # instead of after the whole 3.5MB of weights+x
                for cc in range(CCH):
                    nc.sync.dma_start(
                        out=wq_sb[:, cc, :], in_=wqkv_d[:, cc, :]
                    )
                    nc.sync.dma_start(
                        out=x_t[:, cc, :], in_=xT_d[:, 0, cc, :]
                    )
            else:
                nc.sync.dma_start(out=x_t, in_=xT_d[:, tt, :, :])
            x_tiles.append(x_t)
            if tt == 0:
                # constants that are not needed until later: emit their loads
                # after the first x tile so the first matmul starts sooner
                bias_sb = sb.tile([128, 3], f32, tag="bias")
                nc.sync.dma_start(out=bias_sb, in_=bqkv_d[:, :])
                wp_sb = sb.tile([128, C], f32r, tag="wp")
                nc.sync.dma_start(out=wp_sb, in_=wp_d[:, :])
                ident = sb.tile([128, 128], f32, tag="ident")
                make_identity(nc, ident)
                tri2 = sb.tile([128, 2, 128], f32, tag="tri2")
                make_upper_triangular(nc, tri2[:, 0, :], val=1.0, diag=True)
                nc.gpsimd.tensor_copy(tri2[:, 1, :], tri2[:, 0, :])

        def emit_qkv_tile(tt):
            b = tt // (NT // B)
            tloc = (tt % (NT // B)) * 512
            x_t = x_tiles[tt]
            for g in range(3):
                ps = pa.tile([128, 512], f32, tag="mm")
                for cc in range(CCH):
                    nc.tensor.matmul(
                        ps,
                        wq_sb[:, cc, g * 128 : (g + 1) * 128],
                        x_t[:, cc, :],
                        start=(cc == 0),
                        stop=(cc == CCH - 1),
                    )
                if g == 0:
                    nc.vector.tensor_scalar_add(
                        qT[b][:, tloc : tloc + 512], ps, bias_sb[:, 0:1]
                    )
                elif g == 1:
                    nc.vector.tensor_scalar_add(
                        kT[b][:, tloc : tloc + 512], ps, bias_sb[:, 1:2]
                    )
                else:
                    v_t = vtp.tile([128, 512], f32, tag="v")
                    nc.vector.tensor_scalar_add(v_t, ps, bias_sb[:, 2:3])
                    for j in range(4):
                        pt = pso.tile([128, 128], f32, tag="po")
                        nc.tensor.transpose(pt, v_t[:, j * 128 : (j + 1) * 128], ident)
                        ktl = (tt % (NT // B)) * 4 + j
                        # one strided copy moves both heads' V columns
                        nc.vector.tensor_copy(
                            Vp[b][:, ktl, 0:130].rearrange("p (s e) -> p s e", s=2)[
                                :, :, 0:64
                            ],
                            pt[:, :].rearrange("p (s e) -> p s e", s=2),
                        )

        # ------------- Phase B: attention + output projection -------------
        # Projections are emitted one block behind attention so the PE never
        # waits on the denominator DMA round-trip.
        scale = 1.0 / 8.0  # 1/sqrt(HD)
        pending_proj = []

        def emit_proj(ytb, rcb, b, qb):
            nc.vector.tensor_mul(ytb[0:64, :], ytb[0:64, :], rcb[0:64, :])
            nc.vector.tensor_mul(ytb[64:128, :], ytb[64:128, :], rcb[64:128, :])
            for j in range(4):
                out_t = outp.tile([128, C], f32, tag="out", name="out_t")
                js = slice(j * 128, (j + 1) * 128)
                for ncol in range(2):
                    cs = slice(ncol * 512, (ncol + 1) * 512)
                    pp = pa.tile([128, 512], f32, tag="mm", name="pp")
                    nc.tensor.matmul(
                        pp, ytb[:, js], wp_sb[:, cs], start=True, stop=True
                    )
                    if ncol == 0:
                        nc.scalar.copy(out_t[:, cs], pp)
                    else:
                        nc.vector.tensor_copy(out_t[:, cs], pp)
                row = b * T + qb * 512 + j * 128
                nc.sync.dma_start(out=out_d[row : row + 128, :], in_=out_t)

        def emit_att_block(b, qb):
            if True:  # keep indentation
                n_kt = 4 * (qb + 1)
                po = [
                    pso.tile([65, 512], f32, tag="po", name=f"po{b}{qb}{h}")
                    for h in range(2)
                ]
                pend = []  # att@v pipelined two k-tiles behind S/exp
                for lkt in range(n_kt):
                    r0 = max(0, (lkt - 4 * qb) * 128)
                    ks = slice(lkt * 128, (lkt + 1) * 128)
                    qs = slice(qb * 512 + r0, (qb + 1) * 512)
                    ps2 = pss.tile([128, 1024], f32, tag="s2")
                    nc.tensor.matmul(
                        ps2[:, r0:512], kT[b][0:64, ks], qT[b][0:64, qs],
                        start=True, stop=True,
                    )
                    nc.tensor.matmul(
                        ps2[:, 512 + r0 : 1024], kT[b][64:128, ks], qT[b][64:128, qs],
                        start=True, stop=True,
                    )
                    es = esp.tile([128, 1024], f32r, tag="es")
                    if r0:
                        nc.scalar.activation(
                            es[:, :].rearrange("p (h q) -> p h q", h=2)[:, :, r0:512],
                            ps2[:, :].rearrange("p (h q) -> p h q", h=2)[:, :, r0:512],
                            AF.Exp,
                            scale=scale,
                        )
                    else:
                        nc.scalar.activation(es, ps2, AF.Exp, scale=scale)
                    if lkt >= 4 * qb:  # diagonal tile: causal mask, both heads
                        nc.gpsimd.tensor_mul(
                            es[:, :].rearrange("p (h q) -> p h q", h=2)[
                                :, :, r0 : r0 + 128
                            ],
                            es[:, :].rearrange("p (h q) -> p h q", h=2)[
                                :, :, r0 : r0 + 128
                            ],
                            tri2[:, :, :],
                        )
                    if len(pend) >= 2:
                        for mm in pend.pop(0):
                            nc.tensor.matmul(**mm)
                    pend.append(
                        [
                            dict(
                                out=po[h][:, r0:512],
                                lhsT=Vp[b][:, lkt, h * 65 : (h + 1) * 65],
                                rhs=es[:, h * 512 + r0 : (h + 1) * 512],
                                start=(lkt == 0),
                                stop=(lkt == n_kt - 1),
                            )
                            for h in range(2)
                        ]
                    )
                for grp in pend:
                    for mm in grp:
                        nc.tensor.matmul(**mm)

                # pack un-normalized y now so the PSUM accumulators free up:
                # h0 -> partitions 0-63, h1 -> partitions 64-127 (shifted)
                ytb = ytp.tile([128, 512], f32r, tag="ytb", name="ytb")
                nc.vector.tensor_copy(ytb[0:64, :], po[0][0:64, :])
                nc.vector.tensor_copy(ytb[64:128, :], po[1][0:64, :])
                dh = ytp.tile([65, 1024], f32, tag="dh", name="dh")
                nc.vector.tensor_copy(dh[64:65, 0:512], po[0][64:65, :])
                nc.vector.tensor_copy(dh[64:65, 512:1024], po[1][64:65, :])
                # denominator round-trip: DRAM bounce -> [128,8] transpose ->
                # fast wide reciprocal -> transposed write back -> step-0
                # broadcast reads into a [128, 512] per-query scale tile
                scr = drp.tile([1, 1024], f32, tag="scr", name="scr")
                nc.gpsimd.dma_start(out=scr[0:1, :], in_=dh[64:65, :])
                dn = dnp.tile([128, 8], f32, tag="dn", name="dn")
                nc.gpsimd.dma_start(
                    out=dn, in_=scr[0, :].rearrange("(m p) -> p m", p=128)
                )
                rc = dnp.tile([128, 8], f32, tag="rc", name="rc")
                nc.vector.reciprocal(rc, dn)
                scr2 = drp.tile([1, 1024], f32, tag="scr2", name="scr2")
                nc.gpsimd.dma_start(
                    out=scr2[0, :].rearrange("(m p) -> p m", p=128), in_=rc
                )
                rcb = dnp.tile([128, 512], f32, tag="rcb", name="rcb")
                for h in range(2):
                    src = scr2[0:1, h * 512 : (h + 1) * 512]
                    nc.gpsimd.dma_start(
                        out=rcb[h * 64 : (h + 1) * 64, :],
                        in_=bass.AP(
                            tensor=src.tensor,
                            offset=src.offset,
                            ap=[[0, 64], [1, 512]],
                        ),
                    )
                if pending_proj:
                    emit_proj(*pending_proj.pop())
                pending_proj.append((ytb, rcb, b, qb))

        # Interleaved schedule: batch-0 qkv tiles, then batch-0 attention
        # blocks with batch-1 qkv tiles slotted between them (their DMA and
        # chains hide under attention compute), then batch-1 attention.
        for tt in range(4):
            emit_qkv_tile(tt)
        emit_att_block(0, 0)
        for qb in range(1, 4):
            emit_qkv_tile(3 + qb)
            emit_att_block(0, qb)
        emit_qkv_tile(7)
        for qb in range(4):
            emit_att_block(1, qb)
        emit_proj(*pending_proj.pop())

    nc.finalize()
    _CACHE["nc"] = nc
    return nc


def _prep_inputs(x, w_attn, b_attn, w_proj):
    x = np.ascontiguousarray(np.asarray(x, dtype=np.float32))
    w_attn = np.asarray(w_attn, dtype=np.float32)
    b_attn = np.asarray(b_attn, dtype=np.float32)
    w_proj = np.asarray(w_proj, dtype=np.float32)

    # [p, tt, cc, t] image: xT[p, tt, cc, t] = x_flat[tt*512+t, cc*128+p]
    xT = _round_f32r(
        x.reshape(NT, 512, CCH, 128).transpose(3, 0, 2, 1)
    )
    in_maps = []
    for c in range(NCORE):
        hs = [HPC * c + j for j in range(HPC)]
        blocks = []
        bias_cols = []
        for off in (0, C, 2 * C):
            for h in hs:
                blocks.append(w_attn[:, off + h * HD : off + (h + 1) * HD])
            bias_cols.append(
                np.concatenate([b_attn[off + h * HD : off + (h + 1) * HD] for h in hs])
            )
        wq_flat = _round_f32r(np.concatenate(blocks, axis=1))  # [C, 384]
        wqkv = np.ascontiguousarray(wq_flat.reshape(CCH, 128, 3 * 128).transpose(1, 0, 2))
        bqkv = np.ascontiguousarray(np.stack(bias_cols, axis=1))  # [128, 3]
        wp = _round_f32r(
            np.concatenate([w_proj[h * HD : (h + 1) * HD, :] for h in hs], axis=0)
        )  # [128, C]
        in_maps.append({"xT": xT, "wqkv": wqkv, "bqkv": bqkv, "wp": wp})
    return in_maps


def _run(x, w_attn, b_attn, w_proj, b_proj, trace=False, tmpdir=None):
    from concourse.bass_utils import run_bass_kernel_spmd

    nc = _build()
    in_maps = _prep_inputs(x, w_attn, b_attn, w_proj)
    res = run_bass_kernel_spmd(
        nc, in_maps, list(range(NCORE)), trace=trace, tmpdir=tmpdir
    )
    acc = np.sum(
        np.stack([res.results[i]["out"] for i in range(NCORE)]), axis=0, dtype=np.float64
    )
    out = (acc + np.asarray(b_proj, dtype=np.float64)).astype(np.float32)
    return out.reshape(B, T, C), res


def kernel(x, w_attn, b_attn, w_proj, b_proj):
    out, _ = _run(x, w_attn, b_attn, w_proj, b_proj, trace=False)
    return out



# revision 6
# speedup vs baseline: 1.0597x; 1.0597x over previous
"""Causal self-attention (B=2, T=2048, C=1024, H=16) on 8 trn2 NeuronCores.

Sharding: batch x head-group. Core c handles batch c//4 and the 4 heads
[4*(c%4), 4*(c%4)+4), as two head-pairs A=(h0,h1), B=(h2,h3). Each core
reads only its batch's half of x (8MB instead of 16MB) and writes an
8MB partial output; the host sums 4 partials per batch and adds b_proj.

Per core:
  - QKV projection of its batch (6 groups of 128 weight cols: qA kA vA
    qB kB vB), producing qT/kT in [head_dim, T] layout and V' in
    [T, head_dim] layout via PE transposes, with a ones column per head
    (softmax denominator accumulates during att@V).
  - Flash-style causal attention per (head-pair, q-block): concurrent
    row-group S^T matmul pairs into a 2-bank PSUM tile, one ACT exp for
    both heads, triangular 0/1 mask on diagonal tiles (GpSimd), att@V
    accumulates per head with the denominator in PSUM row 64.
  - Softmax normalization fully on-chip (no DRAM bounce): denominator
    rows are copied to SBUF, partition-broadcast via two K=1 matmuls
    (ones-row stationary, col-group tiling for the second head), DVE
    reciprocal, then a fused normalize-on-copy produces ytb.
  - Output projection contracts 256 y-dims (both head-pairs) per
    128-query chunk; partial [2048, 1024] written to DRAM.
Emission order is tuned for the per-engine FIFO: QKV group chains are
fed between attention k-tiles so ACT's exp stream never starves the PE,
normalization is deferred one block and projection two blocks.

Matmuls run in float32r (fp32 with 12-bit mantissa, 4x faster than fp32
on the PE, full fp32 PSUM accumulate).
"""

import sys

sys.path.insert(0, "/opt/trn_rl_repo")

import numpy as np

B, T, C, H, HD = 2, 2048, 1024, 16, 64
NCORE = 8
HPC = 4           # heads per core
NT = T // 512     # 4 T-tiles (one batch per core)
CCH = C // 128    # 8 contraction chunks
NKT = T // 128    # 16 k-tiles


def _round_f32r(x):
    x = np.ascontiguousarray(x, dtype=np.float32)
    xi = x.view(np.uint32)
    r = (xi + np.uint32(0x7FF) + ((xi >> np.uint32(12)) & np.uint32(1))) & np.uint32(
        0xFFFFF000
    )
    return r.view(np.float32)


_CACHE = {}


def _build():
    if "nc" in _CACHE:
        return _CACHE["nc"]
    from contextlib import ExitStack

    import concourse.bass as bass
    import concourse.bacc as bacc
    import concourse.mybir as mybir
    import concourse.tile as tile
    from concourse.masks import make_identity, make_upper_triangular

    f32, f32r = mybir.dt.float32, mybir.dt.float32r
    AF = mybir.ActivationFunctionType

    nc = bacc.Bacc(None, target_bir_lowering=False, debug=False)
    # x pre-permuted on host to [p, tt, cc, t] so each T-tile DMA reads
    # contiguous runs per partition
    xT_d = nc.dram_tensor("xT", [128, NT, CCH, 512], f32r, kind="ExternalInput")
    wqkv_d = nc.dram_tensor("wqkv", [128, CCH, 6 * 128], f32r, kind="ExternalInput")
    bqkv_d = nc.dram_tensor("bqkv", [128, 6], f32, kind="ExternalInput")
    wp_d = nc.dram_tensor("wp", [128, 2, C], f32r, kind="ExternalInput")
    out_d = nc.dram_tensor("out", [T, C], f32, kind="ExternalOutput")

    with tile.TileContext(nc) as tc, ExitStack() as ctx:
        sb = ctx.enter_context(tc.tile_pool(name="sb", bufs=1))
        xp = ctx.enter_context(tc.tile_pool(name="xp", bufs=2))
        vtp = ctx.enter_context(tc.tile_pool(name="vtp", bufs=2))
        esp = ctx.enter_context(tc.tile_pool(name="esp", bufs=4))
        dnp = ctx.enter_context(tc.tile_pool(name="dnp", bufs=2))
        rcpp = ctx.enter_context(tc.tile_pool(name="rcpp", bufs=2))
        outp = ctx.enter_context(tc.tile_pool(name="outp", bufs=3))
        # PSUM: 2 + 4 + 2 = 8 banks
        pa = ctx.enter_context(tc.tile_pool(name="pa", bufs=2, space="PSUM"))
        pss = ctx.enter_context(tc.tile_pool(name="pss", bufs=2, space="PSUM"))
        pso = ctx.enter_context(tc.tile_pool(name="pso", bufs=2, space="PSUM"))

        wq_sb = sb.tile([128, CCH, 6 * 128], f32r, tag="wq")

        qT = [sb.tile([128, T], f32r, tag=f"qT{hp}", name=f"qT{hp}") for hp in range(2)]
        kT = [sb.tile([128, T], f32r, tag=f"kT{hp}", name=f"kT{hp}") for hp in range(2)]
        # V' per head-pair: per k-tile [V_h0 (64) | 1 | V_h1 (64) | 1] = 130;
        # the ones column accumulates the softmax denominator during att@v
        VW = 130
        Vp = [
            sb.tile([128, NKT, VW], f32r, tag=f"Vp{hp}", name=f"Vp{hp}")
            for hp in range(2)
        ]
        for hp in range(2):
            nc.vector.memset(Vp[hp][:, :, :].bitcast(f32), 1.0)
        # normalized attention outputs, persistent across the two passes
        ytb = [
            [
                sb.tile([128, 512], f32r, tag=f"ytb{hp}{qb}", name=f"ytb{hp}{qb}")
                for qb in range(NT)
            ]
            for hp in range(2)
        ]
        ones_sb = sb.tile([1, 64], f32r, tag="ones")
        nc.vector.memset(ones_sb.bitcast(f32), 1.0)

        # ---------------- QKV projection ----------------
        x_tiles = [None] * NT

        def emit_x_dma(tt):
            x_t = xp.tile([128, CCH, 512], f32r, tag="x", name=f"x{tt}")
            if tt == 0:
                # chunked loads so the first matmul chain starts early
                for cc in range(CCH):
                    nc.sync.dma_start(out=wq_sb[:, cc, :], in_=wqkv_d[:, cc, :])
                    nc.sync.dma_start(out=x_t[:, cc, :], in_=xT_d[:, 0, cc, :])
            else:
                nc.sync.dma_start(out=x_t, in_=xT_d[:, tt, :, :])
            x_tiles[tt] = x_t

        def emit_qkv_group(tt, g):
            # g: 0=qA 1=kA 2=vA 3=qB 4=kB 5=vB
            hp, kind = divmod(g, 3)
            x_t = x_tiles[tt]
            tloc = tt * 512
            ps = pa.tile([128, 512], f32, tag="mm")
            for cc in range(CCH):
                nc.tensor.matmul(
                    ps,
                    wq_sb[:, cc, g * 128 : (g + 1) * 128],
                    x_t[:, cc, :],
                    start=(cc == 0),
                    stop=(cc == CCH - 1),
                )
            if kind == 0:
                nc.vector.tensor_scalar_add(
                    qT[hp][:, tloc : tloc + 512], ps, bias_sb[:, g : g + 1]
                )
            elif kind == 1:
                nc.vector.tensor_scalar_add(
                    kT[hp][:, tloc : tloc + 512], ps, bias_sb[:, g : g + 1]
                )
            else:
                v_t = vtp.tile([128, 512], f32, tag="v")
                nc.vector.tensor_scalar_add(v_t, ps, bias_sb[:, g : g + 1])
                for j in range(4):
                    pt = pa.tile([128, 128], f32, tag="mm")
                    nc.tensor.transpose(pt, v_t[:, j * 128 : (j + 1) * 128], ident)
                    ktl = tt * 4 + j
                    # one strided copy moves both heads' V columns
                    nc.vector.tensor_copy(
                        Vp[hp][:, ktl, 0:130].rearrange("p (s e) -> p s e", s=2)[
                            :, :, 0:64
                        ],
                        pt[:, :].rearrange("p (s e) -> p s e", s=2),
                    )

        emit_x_dma(0)
        # constants not needed immediately: emit loads after the x chunks
        bias_sb = sb.tile([128, 6], f32, tag="bias")
        nc.sync.dma_start(out=bias_sb, in_=bqkv_d[:, :])
        emit_x_dma(1)
        wp_sb = sb.tile([128, 2, C], f32r, tag="wp")
        nc.sync.dma_start(out=wp_sb, in_=wp_d[:, :, :])
        ident = sb.tile([128, 128], f32, tag="ident")
        make_identity(nc, ident)
        tri2 = sb.tile([128, 2, 128], f32, tag="tri2")
        make_upper_triangular(nc, tri2[:, 0, :], val=1.0, diag=True)
        nc.gpsimd.tensor_copy(tri2[:, 1, :], tri2[:, 0, :])
        for g in range(6):
            emit_qkv_group(0, g)

        # ------------- attention + normalization + projection -------------
        scale = 1.0 / 8.0  # 1/sqrt(HD)
        deferred_norm = []
        proj_queue = []

        def make_norm(po, dh, hp, qb):
            def norm():
                # partition-broadcast each head's denominator row via a K=1
                # matmul (ones-row stationary), then reciprocal + fused
                # normalize-on-copy
                den = [pa.tile([64, 512], f32, tag="mm", name="den") for _ in range(2)]
                for h in range(2):
                    nc.tensor.matmul(
                        den[h], ones_sb[0:1, :], dh[0:1, h * 512 : (h + 1) * 512],
                        start=True, stop=True,
                    )
                rcp = rcpp.tile([128, 512], f32, tag="rcp")
                nc.vector.reciprocal(rcp[0:64, :], den[0])
                nc.vector.reciprocal(rcp[64:128, :], den[1])
                yt = ytb[hp][qb]
                nc.vector.tensor_mul(yt[0:64, :], po[0][0:64, :], rcp[0:64, :])
                nc.vector.tensor_mul(yt[64:128, :], po[1][0:64, :], rcp[64:128, :])
            return norm

        def emit_proj(qb):
            for j in range(4):
                out_t = outp.tile([128, C], f32, tag="out", name="out_t")
                js = slice(j * 128, (j + 1) * 128)
                for ncol in range(2):
                    cs = slice(ncol * 512, (ncol + 1) * 512)
                    pp = pa.tile([128, 512], f32, tag="mm", name="pp")
                    nc.tensor.matmul(
                        pp, ytb[0][qb][:, js], wp_sb[:, 0, cs], start=True, stop=False
                    )
                    nc.tensor.matmul(
                        pp, ytb[1][qb][:, js], wp_sb[:, 1, cs], start=False, stop=True
                    )
                    if ncol == 0:
                        nc.scalar.copy(out_t[:, cs], pp)
                    else:
                        nc.vector.tensor_copy(out_t[:, cs], pp)
                row = qb * 512 + j * 128
                nc.sync.dma_start(out=out_d[row : row + 128, :], in_=out_t)

        def emit_att_block(hp, qb, feeds=()):
            feeds = list(feeds)
            n_kt = 4 * (qb + 1)
            po = [
                pso.tile([65, 512], f32, tag="po", name=f"po{hp}{qb}{h}")
                for h in range(2)
            ]
            pend = []  # att@v pipelined two k-tiles behind S/exp
            for lkt in range(n_kt):
                r0 = max(0, (lkt - 4 * qb) * 128)
                ks = slice(lkt * 128, (lkt + 1) * 128)
                qs = slice(qb * 512 + r0, (qb + 1) * 512)
                ps2 = pss.tile([128, 1024], f32, tag="s2")
                nc.tensor.matmul(
                    ps2[:, r0:512], kT[hp][0:64, ks], qT[hp][0:64, qs],
                    start=True, stop=True,
                )
                nc.tensor.matmul(
                    ps2[:, 512 + r0 : 1024], kT[hp][64:128, ks], qT[hp][64:128, qs],
                    start=True, stop=True,
                )
                es = esp.tile([128, 1024], f32r, tag="es")
                if r0:
                    nc.scalar.activation(
                        es[:, :].rearrange("p (h q) -> p h q", h=2)[:, :, r0:512],
                        ps2[:, :].rearrange("p (h q) -> p h q", h=2)[:, :, r0:512],
                        AF.Exp,
                        scale=scale,
                    )
                else:
                    nc.scalar.activation(es, ps2, AF.Exp, scale=scale)
                if lkt >= 4 * qb:  # diagonal tile: causal mask, both heads
                    nc.gpsimd.tensor_mul(
                        es[:, :].rearrange("p (h q) -> p h q", h=2)[
                            :, :, r0 : r0 + 128
                        ],
                        es[:, :].rearrange("p (h q) -> p h q", h=2)[
                            :, :, r0 : r0 + 128
                        ],
                        tri2[:, :, :],
                    )
                if lkt == 0 and deferred_norm:
                    deferred_norm.pop()()
                if lkt == 1 and len(proj_queue) >= 2:
                    emit_proj(proj_queue.pop(0))
                if feeds:
                    feeds.pop(0)()
                if len(pend) >= 2:
                    for mm in pend.pop(0):
                        nc.tensor.matmul(**mm)
                pend.append(
                    [
                        dict(
                            out=po[h][:, r0:512],
                            lhsT=Vp[hp][:, lkt, h * 65 : (h + 1) * 65],
                            rhs=es[:, h * 512 + r0 : (h + 1) * 512],
                            start=(lkt == 0),
                            stop=(lkt == n_kt - 1),
                        )
                        for h in range(2)
                    ]
                )
            for grp in pend:
                for mm in grp:
                    nc.tensor.matmul(**mm)
            for f in feeds:  # leftover feeds (short blocks)
                f()
            # denominator rows -> SBUF; the broadcast + reciprocal + fused
            # normalize run at the start of the NEXT block (per-engine FIFO:
            # lets the next block's S matmuls issue first)
            dh = dnp.tile([1, 1024], f32r, tag="dh", name="dh")
            nc.vector.tensor_copy(dh[0:1, 0:512], po[0][64:65, :])
            nc.vector.tensor_copy(dh[0:1, 512:1024], po[1][64:65, :])
            deferred_norm.append(make_norm(po, dh, hp, qb))

        # Schedule: pass A ascending (attention starts right after the first
        # qkv tile; later qkv tiles are fed between its k-tiles), pass B
        # descending (tail block is the smallest). Projections are deferred
        # two blocks so their PE ops never wait on fresh normalizations.
        def feed_funcs(tt, with_dma):
            fs = []
            if with_dma:
                fs.append(lambda tt=tt: emit_x_dma(tt))
            for g in range(6):
                fs.append(lambda tt=tt, g=g: emit_qkv_group(tt, g))
            return fs

        emit_att_block(0, 0, feeds=feed_funcs(1, False) + [lambda: emit_x_dma(2)])
        emit_att_block(0, 1, feeds=[lambda: emit_x_dma(3)] + feed_funcs(2, False))
        emit_att_block(0, 2, feeds=feed_funcs(3, False))
        emit_att_block(0, 3)
        for qb in (3, 2, 1, 0):
            emit_att_block(1, qb)
            proj_queue.append(qb)
        while deferred_norm:
            deferred_norm.pop()()
        while proj_queue:
            emit_proj(proj_queue.pop(0))

    nc.finalize()
    _CACHE["nc"] = nc
    return nc


def _prep_inputs(x, w_attn, b_attn, w_proj):
    x = np.ascontiguousarray(np.asarray(x, dtype=np.float32))
    w_attn = np.asarray(w_attn, dtype=np.float32)
    b_attn = np.asarray(b_attn, dtype=np.float32)
    w_proj = np.asarray(w_proj, dtype=np.float32)

    # per batch: xT[p, tt, cc, t] = x[b, tt*512+t, cc*128+p]
    xTs = [
        _round_f32r(x[b].reshape(NT, 512, CCH, 128).transpose(3, 0, 2, 1))
        for b in range(B)
    ]
    in_maps = []
    for c in range(NCORE):
        b = c // 4
        hq = (c % 4) * HPC  # first global head on this core
        blocks = []
        bias_cols = []
        for hp in range(2):
            hs = [hq + 2 * hp, hq + 2 * hp + 1]
            for off in (0, C, 2 * C):  # q, k, v
                for h in hs:
                    blocks.append(w_attn[:, off + h * HD : off + (h + 1) * HD])
                bias_cols.append(
                    np.concatenate(
                        [b_attn[off + h * HD : off + (h + 1) * HD] for h in hs]
                    )
                )
        wq_flat = _round_f32r(np.concatenate(blocks, axis=1))  # [C, 768]
        wqkv = np.ascontiguousarray(
            wq_flat.reshape(CCH, 128, 6 * 128).transpose(1, 0, 2)
        )
        bqkv = np.ascontiguousarray(np.stack(bias_cols, axis=1))  # [128, 6]
        wp = _round_f32r(
            w_proj[hq * HD : hq * HD + 256, :].reshape(2, 128, C).transpose(1, 0, 2)
        )  # [128, 2, C]
        in_maps.append({"xT": xTs[b], "wqkv": wqkv, "bqkv": bqkv, "wp": wp})
    return in_maps


def _run(x, w_attn, b_attn, w_proj, b_proj, trace=False, tmpdir=None):
    from concourse.bass_utils import run_bass_kernel_spmd

    nc = _build()
    in_maps = _prep_inputs(x, w_attn, b_attn, w_proj)
    res = run_bass_kernel_spmd(
        nc, in_maps, list(range(NCORE)), trace=trace, tmpdir=tmpdir
    )
    bp = np.asarray(b_proj, dtype=np.float64)
    outs = []
    for b in range(B):
        acc = np.sum(
            np.stack([res.results[b * 4 + i]["out"] for i in range(4)]),
            axis=0,
            dtype=np.float64,
        )
        outs.append((acc + bp).astype(np.float32))
    return np.stack(outs), res


def kernel(x, w_attn, b_attn, w_proj, b_proj):
    out, _ = _run(x, w_attn, b_attn, w_proj, b_proj, trace=False)
    return out


# revision 16
# speedup vs baseline: 1.3231x; 1.2485x over previous
"""Causal self-attention (B=2, T=2048, C=1024, H=16) on 8 trn2 NeuronCores.

Sharding: batch x head-group. Core c handles batch c//4 and the 4 heads
[4*(c%4), 4*(c%4)+4), as two head-pairs A=(h0,h1), B=(h2,h3). Each core
reads only its batch's half of x (8MB instead of 16MB) and writes an
8MB partial output; the host sums 4 partials per batch and adds b_proj.

Per core:
  - QKV projection of its batch (6 groups of 128 weight cols: qA kA vA
    qB kB vB), producing qT/kT in [head_dim, T] layout and V' in
    [T, head_dim] layout via PE transposes, with a ones column per head
    (softmax denominator accumulates during att@V).
  - Flash-style causal attention per (head-pair, q-block): concurrent
    row-group S^T matmul pairs into a 2-bank PSUM tile, one ACT exp for
    both heads, triangular 0/1 mask on diagonal tiles (GpSimd), att@V
    accumulates per head with the denominator in PSUM row 64.
  - Softmax normalization fully on-chip (no DRAM bounce): denominator
    rows are copied to SBUF, partition-broadcast via two K=1 matmuls
    (ones-row stationary, col-group tiling for the second head), DVE
    reciprocal, then a fused normalize-on-copy produces ytb.
  - Output projection contracts 256 y-dims (both head-pairs) per
    128-query chunk; partial [2048, 1024] written to DRAM.
Emission order is tuned for the per-engine FIFO: QKV group chains are
fed between attention k-tiles so ACT's exp stream never starves the PE,
normalization is deferred one block and projection two blocks.

Matmuls run in float32r (fp32 with 12-bit mantissa, 4x faster than fp32
on the PE, full fp32 PSUM accumulate).
"""

import sys

sys.path.insert(0, "/opt/trn_rl_repo")

import numpy as np

B, T, C, H, HD = 2, 2048, 1024, 16, 64
NCORE = 8
HPC = 4           # heads per core
NT = T // 512     # 4 T-tiles (one batch per core)
CCH = C // 128    # 8 contraction chunks
NKT = T // 128    # 16 k-tiles


def _round_f32r(x):
    x = np.ascontiguousarray(x, dtype=np.float32)
    xi = x.view(np.uint32)
    r = (xi + np.uint32(0x7FF) + ((xi >> np.uint32(12)) & np.uint32(1))) & np.uint32(
        0xFFFFF000
    )
    return r.view(np.float32)


_CACHE = {}


def _build():
    if "nc" in _CACHE:
        return _CACHE["nc"]
    from contextlib import ExitStack

    import concourse.bass as bass
    import concourse.bacc as bacc
    import concourse.mybir as mybir
    import concourse.tile as tile
    from concourse.masks import make_identity, make_upper_triangular

    f32, f32r = mybir.dt.float32, mybir.dt.float32r
    AF = mybir.ActivationFunctionType

    nc = bacc.Bacc(None, target_bir_lowering=False, debug=False)
    # x pre-permuted on host to [p, tt, cc, t] so each T-tile DMA reads
    # contiguous runs per partition
    xT_d = nc.dram_tensor("xT", [128, NT, CCH, 512], f32r, kind="ExternalInput")
    wqkv_d = nc.dram_tensor("wqkv", [128, CCH, 6 * 128], f32r, kind="ExternalInput")
    bqkv_d = nc.dram_tensor("bqkv", [128, 6], f32, kind="ExternalInput")
    wp_d = nc.dram_tensor("wp", [128, 2, C], f32r, kind="ExternalInput")
    sel_d = nc.dram_tensor("sel", [33, 128], f32r, kind="ExternalInput")
    out_d = nc.dram_tensor("out", [T, C], f32, kind="ExternalOutput")

    with tile.TileContext(nc) as tc, ExitStack() as ctx:
        sb = ctx.enter_context(tc.tile_pool(name="sb", bufs=1))
        xp = ctx.enter_context(tc.tile_pool(name="xp", bufs=2))
        vtp = ctx.enter_context(tc.tile_pool(name="vtp", bufs=2))
        esp = ctx.enter_context(tc.tile_pool(name="esp", bufs=4))
        dnp = ctx.enter_context(tc.tile_pool(name="dnp", bufs=2))
        rcpp = ctx.enter_context(tc.tile_pool(name="rcpp", bufs=2))
        outp = ctx.enter_context(tc.tile_pool(name="outp", bufs=3))
        # PSUM: 2 + 4 + 2 = 8 banks
        pa = ctx.enter_context(tc.tile_pool(name="pa", bufs=2, space="PSUM"))
        pss = ctx.enter_context(tc.tile_pool(name="pss", bufs=2, space="PSUM"))
        pso = ctx.enter_context(tc.tile_pool(name="pso", bufs=2, space="PSUM"))

        wq_sb = sb.tile([128, CCH, 6 * 128], f32r, tag="wq")

        qT = [sb.tile([128, T], f32r, tag=f"qT{hp}", name=f"qT{hp}") for hp in range(2)]
        kT = [sb.tile([128, T], f32r, tag=f"kT{hp}", name=f"kT{hp}") for hp in range(2)]
        # V' per head-pair: per k-tile [V_h0 (64) | 1 | V_h1 (64) | 1] = 130;
        # the ones column accumulates the softmax denominator during att@v
        VW = 130
        Vp = [
            sb.tile([128, NKT, VW], f32r, tag=f"Vp{hp}", name=f"Vp{hp}")
            for hp in range(2)
        ]
        for hp in range(2):
            nc.vector.memset(Vp[hp][:, :, :].bitcast(f32), 1.0)
        # normalized attention outputs, persistent across the two passes
        ytb = [
            [
                sb.tile([128, 512], f32r, tag=f"ytb{hp}{qb}", name=f"ytb{hp}{qb}")
                for qb in range(NT)
            ]
            for hp in range(2)
        ]
        # selector [2, 128]: col j reads partition 0 (head0 denom) for j<64,
        # partition 1 (head1 denom) for j>=64 -- one K=2 matmul broadcasts
        # both heads' denominator rows into a single [128, 512] PSUM bank
        sel_sb = sb.tile([33, 128], f32r, tag="sel")
        nc.sync.dma_start(out=sel_sb, in_=sel_d[:, :])

        # ---------------- QKV projection ----------------
        x_tiles = [None] * NT

        def emit_x_dma(tt):
            x_t = xp.tile([128, CCH, 512], f32r, tag="x", name=f"x{tt}")
            if tt == 0:
                # chunked loads so the first matmul chain starts early
                for cc in range(CCH):
                    nc.sync.dma_start(out=wq_sb[:, cc, :], in_=wqkv_d[:, cc, :])
                    nc.sync.dma_start(out=x_t[:, cc, :], in_=xT_d[:, 0, cc, :])
            else:
                nc.sync.dma_start(out=x_t, in_=xT_d[:, tt, :, :])
            x_tiles[tt] = x_t

        def emit_qkv_group(tt, g):
            # g: 0=qA 1=kA 2=vA 3=qB 4=kB 5=vB
            hp, kind = divmod(g, 3)
            x_t = x_tiles[tt]
            tloc = tt * 512
            ps = pa.tile([128, 512], f32, tag="mm")
            for cc in range(CCH):
                nc.tensor.matmul(
                    ps,
                    wq_sb[:, cc, g * 128 : (g + 1) * 128],
                    x_t[:, cc, :],
                    start=(cc == 0),
                    stop=(cc == CCH - 1),
                )
            if kind == 0:
                nc.vector.tensor_scalar_add(
                    qT[hp][:, tloc : tloc + 512], ps, bias_sb[:, g : g + 1]
                )
            elif kind == 1:
                nc.vector.tensor_scalar_add(
                    kT[hp][:, tloc : tloc + 512], ps, bias_sb[:, g : g + 1]
                )
            else:
                v_t = vtp.tile([128, 512], f32, tag="v")
                nc.vector.tensor_scalar_add(v_t, ps, bias_sb[:, g : g + 1])
                for j in range(4):
                    pt = pa.tile([128, 128], f32, tag="mm")
                    nc.tensor.transpose(pt, v_t[:, j * 128 : (j + 1) * 128], ident)
                    ktl = tt * 4 + j
                    # one strided copy moves both heads' V columns
                    nc.vector.tensor_copy(
                        Vp[hp][:, ktl, 0:130].rearrange("p (s e) -> p s e", s=2)[
                            :, :, 0:64
                        ],
                        pt[:, :].rearrange("p (s e) -> p s e", s=2),
                    )

        emit_x_dma(0)
        # constants not needed immediately: emit loads after the x chunks
        bias_sb = sb.tile([128, 6], f32, tag="bias")
        nc.sync.dma_start(out=bias_sb, in_=bqkv_d[:, :])
        emit_x_dma(1)
        wp_sb = sb.tile([128, 2, C], f32r, tag="wp")
        nc.sync.dma_start(out=wp_sb, in_=wp_d[:, :, :])
        ident = sb.tile([128, 128], f32, tag="ident")
        make_identity(nc, ident)
        tri2 = sb.tile([128, 2, 128], f32, tag="tri2")
        make_upper_triangular(nc, tri2[:, 0, :], val=1.0, diag=True)
        nc.gpsimd.tensor_copy(tri2[:, 1, :], tri2[:, 0, :])
        # PE warm-up: junk matmuls during the initial DMA wait pull the HAM
        # clock gate to 8/8 before the first real qkv chain issues
        wu = pa.tile([128, 128], f32, tag="mm", name="warmup")
        for _ in range(16):
            nc.tensor.matmul(wu, ident, ident, start=True, stop=True)
        for g in range(6):
            emit_qkv_group(0, g)

        # ------------- attention + normalization + projection -------------
        scale = 1.0 / 8.0  # 1/sqrt(HD)
        deferred_norm = []
        proj_queue = []

        def make_norm(dh, hp, qb):
            def norm():
                # broadcast both heads' denominators into one PSUM bank via a
                # single K=2 matmul (selector stationary), one reciprocal,
                # then normalize ytb in place. Runs lazily one block later --
                # nothing here holds PSUM po banks.
                den = pa.tile([128, 512], f32, tag="mm", name="den")
                nc.tensor.matmul(den, sel_sb[0:33, :], dh[0:33, :], start=True, stop=True)
                rcp = rcpp.tile([128, 512], f32, tag="rcp")
                nc.vector.reciprocal(rcp, den)
                yt = ytb[hp][qb]
                nc.vector.tensor_mul(yt[0:64, :], yt[0:64, :], rcp[0:64, :])
                nc.vector.tensor_mul(yt[64:128, :], yt[64:128, :], rcp[64:128, :])
            return norm

        def emit_proj(qb):
            for j in range(4):
                out_t = outp.tile([128, C], f32, tag="out", name="out_t")
                js = slice(j * 128, (j + 1) * 128)
                for ncol in range(2):
                    cs = slice(ncol * 512, (ncol + 1) * 512)
                    pp = pa.tile([128, 512], f32, tag="mm", name="pp")
                    nc.tensor.matmul(
                        pp, ytb[0][qb][:, js], wp_sb[:, 0, cs], start=True, stop=False
                    )
                    nc.tensor.matmul(
                        pp, ytb[1][qb][:, js], wp_sb[:, 1, cs], start=False, stop=True
                    )
                    if ncol == 0:
                        nc.scalar.copy(out_t[:, cs], pp)
                    else:
                        nc.vector.tensor_copy(out_t[:, cs], pp)
                row = qb * 512 + j * 128
                nc.sync.dma_start(out=out_d[row : row + 128, :], in_=out_t)

        def emit_att_block(hp, qb, feeds=()):
            feeds = list(feeds)
            n_kt = 4 * (qb + 1)
            po = [
                pso.tile([65, 512], f32, tag="po", name=f"po{hp}{qb}{h}")
                for h in range(2)
            ]
            pend = []  # att@v pipelined two k-tiles behind S/exp
            for lkt in range(n_kt):
                r0 = max(0, (lkt - 4 * qb) * 128)
                ks = slice(lkt * 128, (lkt + 1) * 128)
                qs = slice(qb * 512 + r0, (qb + 1) * 512)
                ps2 = pss.tile([128, 1024], f32, tag="s2")
                nc.tensor.matmul(
                    ps2[:, r0:512], kT[hp][0:64, ks], qT[hp][0:64, qs],
                    start=True, stop=True,
                )
                nc.tensor.matmul(
                    ps2[:, 512 + r0 : 1024], kT[hp][64:128, ks], qT[hp][64:128, qs],
                    start=True, stop=True,
                )
                es = esp.tile([128, 1024], f32r, tag="es")
                if r0:
                    nc.scalar.activation(
                        es[:, :].rearrange("p (h q) -> p h q", h=2)[:, :, r0:512],
                        ps2[:, :].rearrange("p (h q) -> p h q", h=2)[:, :, r0:512],
                        AF.Exp,
                        scale=scale,
                    )
                else:
                    nc.scalar.activation(es, ps2, AF.Exp, scale=scale)
                if lkt >= 4 * qb:  # diagonal tile: causal mask, both heads
                    nc.gpsimd.tensor_mul(
                        es[:, :].rearrange("p (h q) -> p h q", h=2)[
                            :, :, r0 : r0 + 128
                        ],
                        es[:, :].rearrange("p (h q) -> p h q", h=2)[
                            :, :, r0 : r0 + 128
                        ],
                        tri2[:, :, :],
                    )
                if lkt == 0 and deferred_norm:
                    deferred_norm.pop()()
                if lkt == 1 and len(proj_queue) >= 2:
                    emit_proj(proj_queue.pop(0))
                if feeds:
                    feeds.pop(0)()
                if len(pend) >= 2:
                    for mm in pend.pop(0):
                        nc.tensor.matmul(**mm)
                pend.append(
                    [
                        dict(
                            out=po[h][:, r0:512],
                            lhsT=Vp[hp][:, lkt, h * 65 : (h + 1) * 65],
                            rhs=es[:, h * 512 + r0 : (h + 1) * 512],
                            start=(lkt == 0),
                            stop=(lkt == n_kt - 1),
                        )
                        for h in range(2)
                    ]
                )
            for grp in pend:
                for mm in grp:
                    nc.tensor.matmul(**mm)
            for f in feeds:  # leftover feeds (short blocks)
                f()
            # evacuate po fast so the next block's att@v gets its PSUM banks:
            # denominator rows -> SBUF partitions 0/1 on ACT, un-normalized y
            # -> ytb on DVE (reciprocal + normalize run lazily, a block later)
            dh = dnp.tile([33, 512], f32r, tag="dh", name="dh")
            yt = ytb[hp][qb]
            nc.scalar.copy(dh[0:1, :], po[0][64:65, :])
            nc.vector.tensor_copy(yt[0:64, :], po[0][0:64, :])
            nc.scalar.copy(dh[32:33, :], po[1][64:65, :])
            nc.vector.tensor_copy(yt[64:128, :], po[1][0:64, :])
            deferred_norm.append(make_norm(dh, hp, qb))

        # Schedule: pass A ascending (attention starts right after the first
        # qkv tile; later qkv tiles are fed between its k-tiles), pass B
        # descending (tail block is the smallest). Projections are deferred
        # two blocks so their PE ops never wait on fresh normalizations.
        def feed_funcs(tt, with_dma):
            fs = []
            if with_dma:
                fs.append(lambda tt=tt: emit_x_dma(tt))
            for g in range(6):
                fs.append(lambda tt=tt, g=g: emit_qkv_group(tt, g))
            return fs

        emit_att_block(0, 0, feeds=feed_funcs(1, False) + [lambda: emit_x_dma(2)])
        emit_att_block(0, 1, feeds=[lambda: emit_x_dma(3)] + feed_funcs(2, False))
        emit_att_block(0, 2, feeds=feed_funcs(3, False))
        emit_att_block(0, 3)
        for qb in (3, 2, 1, 0):
            emit_att_block(1, qb)
            proj_queue.append(qb)
        while deferred_norm:
            deferred_norm.pop()()
        while proj_queue:
            emit_proj(proj_queue.pop(0))

    nc.finalize()
    _CACHE["nc"] = nc
    return nc


def _prep_inputs(x, w_attn, b_attn, w_proj):
    x = np.ascontiguousarray(np.asarray(x, dtype=np.float32))
    w_attn = np.asarray(w_attn, dtype=np.float32)
    b_attn = np.asarray(b_attn, dtype=np.float32)
    w_proj = np.asarray(w_proj, dtype=np.float32)

    # per batch: xT[p, tt, cc, t] = x[b, tt*512+t, cc*128+p]
    xTs = [
        _round_f32r(x[b].reshape(NT, 512, CCH, 128).transpose(3, 0, 2, 1))
        for b in range(B)
    ]
    in_maps = []
    for c in range(NCORE):
        b = c // 4
        hq = (c % 4) * HPC  # first global head on this core
        blocks = []
        bias_cols = []
        for hp in range(2):
            hs = [hq + 2 * hp, hq + 2 * hp + 1]
            for off in (0, C, 2 * C):  # q, k, v
                for h in hs:
                    blocks.append(w_attn[:, off + h * HD : off + (h + 1) * HD])
                bias_cols.append(
                    np.concatenate(
                        [b_attn[off + h * HD : off + (h + 1) * HD] for h in hs]
                    )
                )
        wq_flat = _round_f32r(np.concatenate(blocks, axis=1))  # [C, 768]
        wqkv = np.ascontiguousarray(
            wq_flat.reshape(CCH, 128, 6 * 128).transpose(1, 0, 2)
        )
        bqkv = np.ascontiguousarray(np.stack(bias_cols, axis=1))  # [128, 6]
        wp = _round_f32r(
            w_proj[hq * HD : hq * HD + 256, :].reshape(2, 128, C).transpose(1, 0, 2)
        )  # [128, 2, C]
        sel = np.zeros((33, 128), dtype=np.float32)
        sel[0, 0:64] = 1.0
        sel[32, 64:128] = 1.0
        in_maps.append(
            {"xT": xTs[b], "wqkv": wqkv, "bqkv": bqkv, "wp": wp, "sel": sel}
        )
    return in_maps


def _run(x, w_attn, b_attn, w_proj, b_proj, trace=False, tmpdir=None):
    from concourse.bass_utils import run_bass_kernel_spmd

    nc = _build()
    in_maps = _prep_inputs(x, w_attn, b_attn, w_proj)
    res = run_bass_kernel_spmd(
        nc, in_maps, list(range(NCORE)), trace=trace, tmpdir=tmpdir
    )
    bp = np.asarray(b_proj, dtype=np.float64)
    outs = []
    for b in range(B):
        acc = np.sum(
            np.stack([res.results[b * 4 + i]["out"] for i in range(4)]),
            axis=0,
            dtype=np.float64,
        )
        outs.append((acc + bp).astype(np.float32))
    return np.stack(outs), res


def kernel(x, w_attn, b_attn, w_proj, b_proj):
    out, _ = _run(x, w_attn, b_attn, w_proj, b_proj, trace=False)
    return out
